# revision 2
# baseline (speedup 1.0000x reference)
"""FFM layer kernel for 8 Trainium2 NeuronCores — fp8 DoubleRow version.

Math (reference): x[B,39] = 13 dense cols + 26 sparse index cols (ints 0..99
stored as f32).  inputs[B,2613] = [dense | one_hot(sparse)], then
  linear = inputs @ w.T + b
  field  = einsum('bn,nfk->bfk', inputs, v)        # [B,39,16]
  cross  = 0.5*sum_k((sum_f field)^2 - sum_f field^2)
  out    = sigmoid(linear + cross)

Strategy: data-parallel over batch, 2048 rows/core.  The one-hot matrix is
exactly representable in fp8 (0/1), so the host prebakes it as fp8 bytes —
the same DMA volume as shipping raw indices — and the PE runs DoubleRow fp8
matmuls that contract K=256 per pass: 11 chunks instead of fp16's 21, at
~218ns per 512-col matmul (vs 213ns for a K=128 fp16 pass).
  psum[128b, 658] = sum_c ohT_c[128,2,128b].T(DR) @ vperm_c[128,2,658]
Feature rows r = 256c + 128j + p: [bias | 13 x_hi | 13 x_lo | pad to 32 |
26*100 one-hot | zero tail].  x_hi = fp8(x), x_lo = (x - x_hi)*64 paired
with v/64 rows recovers dense precision lost to fp8's 3-bit mantissa.

Precision: fp8's 3-bit mantissa is fine for sum_f field^2 (624 independent
quantization errors cancel) but not for (sum_f field)^2 or the linear term,
where errors correlate through the sum before squaring.  So s_k and linear
get dedicated hi/lo fp8 column pairs (cols 624-657): V1hi = fp8(512*sum_f v)
plus a x32 residual, giving ~fp16-equivalent precision from the same fp8
matmul, and eliminating the 624-wide DVE row-reduce entirely.  Field cols
carry a 1024x scale (into e4m3 normal range); all descales fold into the
epilogue activation scale factors.  Epilogue per batch tile: 3 tiny DVE
ops + square-accumulate + sigmoid on the scalar engine.  Throwaway DR
warmup matmuls during the DMA head ramp the PE clock (1.2GHz cold ->
2.4GHz after ~3.4us busy) so the real matmuls start at full speed.
"""

import math
import sys

sys.path.insert(0, "/opt/trn_rl_repo")

import numpy as np
import ml_dtypes

import concourse.tile as tile
from concourse import bacc, mybir
from concourse.bass_utils import run_bass_kernel_spmd

N_CORES = 8
B_FULL = 16384
BC = B_FULL // N_CORES  # 2048 rows per core
P = 128
N_DENSE = 13
N_SPARSE = 26
SPARSE_DIM = 100
N_FIELD = 39
K_DIM = 16
NCHUNK = 11
K2 = 2 * P              # 256 contraction rows per DoubleRow chunk
RTOT = NCHUNK * K2      # 2816 padded feature rows
SP0 = 32                # first one-hot row
NFEAT_END = SP0 + N_SPARSE * SPARSE_DIM  # 2632
CF = N_FIELD * K_DIM    # 624 field cols (k-major, f-minor)
# cols 624..639: V1hi (s[b,k] high part), 640..655: V1res, 656: lin hi,
# 657: lin res
NCOL = CF + 2 * K_DIM + 2  # 658 real cols
CPAD = 672              # pad so the DR pair-stride is 16B aligned
GB = 4                  # batch tiles per group (4 psum tiles = 8 banks)
VSCALE = 1024.0         # field cols scale: lifts v into e4m3 normal range
S1 = 512.0              # s/lin hi+res cols scale (res at same scale: fp8
                        # precision is relative, so no extra factor needed)
XLO_S = 64.0            # x residual rows scale

F8 = mybir.dt.float8e4
F16 = mybir.dt.float16
F32 = mybir.dt.float32
NP_F8 = ml_dtypes.float8_e4m3
DR = mybir.MatmulPerfMode.DoubleRow

_prog_cache = {}


def _build_program(bc):
    """One SPMD program for a batch slice of `bc` rows (all cores identical)."""
    nbt = bc // P
    ngroups = nbt // GB
    assert nbt % GB == 0
    gw = GB * P  # one-hot column width DMA'd per group

    nc = bacc.Bacc("TRN2", target_bir_lowering=False, debug=False)
    # host layouts are partition-major with long contiguous per-partition
    # runs (short descriptor runs throttle the DMA engines)
    oh_d = nc.declare_dram_parameter(
        "oh", [ngroups, P, NCHUNK, 2, gw], F8, isOutput=False)
    vp_d = nc.declare_dram_parameter(
        "vperm", [P, NCHUNK, 2, CPAD], F8, isOutput=False)
    y_d = nc.declare_dram_parameter("y", [P, nbt], F32, isOutput=True)

    # oh chunk sub-ranges per group: tiny first sub so chunk 0 lands early,
    # alternating HWDGE queues so descriptor generation overlaps
    OSUB = [(0, 1), (1, 4), (4, 8), (8, NCHUNK)]
    OSUB_ENG = ("sync", "scalar", "sync", "scalar")
    VSUB = [(0, 2), (2, 7), (7, NCHUNK)]
    VSUB_ENG = ("scalar", "sync", "scalar")

    with tile.TileContext(nc) as tc:
        with (
            tc.tile_pool(name="pers", bufs=1) as pers,
            tc.tile_pool(name="psum", bufs=4, space="PSUM") as psum,
            tc.tile_pool(name="epi", bufs=3) as epi,
        ):
            oh_all = pers.tile([P, ngroups, NCHUNK, 2, gw], F8, tag="oh")
            vp_all = pers.tile([P, NCHUNK, 2, CPAD], F8, tag="vp")
            y_all = pers.tile([P, nbt], F32, tag="yall")

            def load_oh(g, subs=OSUB, engs=OSUB_ENG):
                # group-major SBUF layout: src and dst are both contiguous
                # per partition, so the DMA runs at full ring throughput
                for (lo, hi), ename in zip(subs, engs):
                    eng = getattr(nc, ename)
                    eng.dma_start(
                        oh_all[:, g, lo:hi, :, :],
                        oh_d[g, :, lo:hi, :, :])

            def load_vp(lo, hi, ename):
                getattr(nc, ename).dma_start(
                    vp_all[:, lo:hi, :, :], vp_d[:, lo:hi, :, :])

            # vp chunks 0-1 first: they gate the first matmuls.  The head
            # is DMA-bandwidth-bound: compute can start only once vp and
            # the group-0 one-hot (~3.2MB) have streamed in (~15us wall);
            # the warmup bridges that window and ramps the PE clock.
            load_vp(*VSUB[0], VSUB_ENG[0])
            load_oh(0)
            for (lo, hi), ename in list(zip(VSUB, VSUB_ENG))[1:]:
                load_vp(lo, hi, ename)
            if ngroups > 1:
                load_oh(1)

            # PE warmup: throwaway DR matmuls on zeroed tiles during the
            # DMA head release the HAM clock throttle (1.2GHz cold ->
            # 2.4GHz after ~3.4us busy) so the real matmuls start at full
            # speed, sized to end about when the group-0 DMA lands
            wz_a = pers.tile([P, 2, P], F8, tag="wza")
            wz_b = pers.tile([P, 2, 512], F8, tag="wzb")
            nc.vector.memset(wz_a[:], 0.0)
            nc.vector.memset(wz_b[:], 0.0)
            wps = psum.tile([P, CPAD], F32, tag="ps", name="warmps")
            for _ in range(8):
                nc.tensor.matmul(wps[:, 0:512], wz_a[:], wz_b[:],
                                 start=True, stop=True, perf_mode=DR)
            for _ in range(28):
                nc.tensor.matmul(wps[:, 0:64], wz_a[:], wz_b[:, :, 0:64],
                                 start=True, stop=True, perf_mode=DR)

            for g in range(ngroups):
                if g + 2 < ngroups:
                    load_oh(g + 2)
                for b4 in range(GB):
                    bt = g * GB + b4
                    ps = psum.tile([P, CPAD], F32, tag="ps")
                    for c in range(NCHUNK):
                        lhs = oh_all[:, g, c, :, b4 * P:(b4 + 1) * P]
                        nc.tensor.matmul(
                            ps[:, 0:512], lhs, vp_all[:, c, :, 0:512],
                            start=(c == 0), stop=(c == NCHUNK - 1),
                            perf_mode=DR,
                        )
                        nc.tensor.matmul(
                            ps[:, 512:NCOL], lhs, vp_all[:, c, :, 512:NCOL],
                            start=(c == 0), stop=(c == NCHUNK - 1),
                            perf_mode=DR,
                        )
                    # epilogue: combine hi/res column pairs (same scale, so
                    # a strided pair-reduce does it — DVE reads only one
                    # PSUM operand per op), square-accumulate, sigmoid.
                    # psum carries VSCALE (field) / S1 (s, lin) scales,
                    # folded into activation scale factors; the 0.5 cross
                    # factor hides in the 1/sqrt(2) of each Square.
                    s_t = epi.tile([P, K_DIM], F32, tag="s")
                    nc.vector.tensor_reduce(
                        out=s_t[:],
                        in_=ps[:, CF:CF + 2 * K_DIM].rearrange(
                            "p (two k) -> p k two", two=2),
                        axis=mybir.AxisListType.X,
                        op=mybir.AluOpType.add,
                    )
                    lin5 = epi.tile([P, 1], F32, tag="lin5")
                    nc.vector.tensor_reduce(
                        out=lin5[:], in_=ps[:, NCOL - 2:NCOL],
                        axis=mybir.AxisListType.X,
                        op=mybir.AluOpType.add,
                    )
                    sq_scr = epi.tile([P, CF], F32, tag="sqscr")
                    sqsum = epi.tile([P, 1], F32, tag="sqsum")
                    nc.scalar.activation(
                        out=sq_scr[:], in_=ps[:, 0:CF],
                        func=mybir.ActivationFunctionType.Square,
                        scale=1.0 / (VSCALE * math.sqrt(2.0)),
                        accum_out=sqsum[:],
                    )
                    # b2 = lin - 0.5*q off the critical path: the final
                    # chain is then s -> square-accum -> sigmoid only
                    b2_t = epi.tile([P, 1], F32, tag="b2")
                    nc.vector.scalar_tensor_tensor(
                        out=b2_t[:], in0=lin5[:], scalar=1.0 / S1,
                        in1=sqsum[:],
                        op0=mybir.AluOpType.mult,
                        op1=mybir.AluOpType.subtract,
                    )
                    s2_scr = epi.tile([P, K_DIM], F32, tag="s2scr")
                    s2sum = epi.tile([P, 1], F32, tag="s2sum")
                    nc.scalar.activation(
                        out=s2_scr[:], in_=s_t[:],
                        func=mybir.ActivationFunctionType.Square,
                        scale=1.0 / (S1 * math.sqrt(2.0)),
                        accum_out=s2sum[:],
                    )
                    nc.scalar.activation(
                        out=y_all[:, bt:bt + 1], in_=s2sum[:],
                        func=mybir.ActivationFunctionType.Sigmoid,
                        scale=1.0, bias=b2_t[:],
                    )
                # per-group output DMA so only the last group's 2KB is on
                # the tail critical path; on sync, which is idle by then
                # (Act's in-order stream would delay the submit)
                nc.sync.dma_start(y_d[:, g * GB:(g + 1) * GB],
                                  y_all[:, g * GB:(g + 1) * GB])

    nc.compile()
    return nc


def _get_program(bc):
    if bc not in _prog_cache:
        _prog_cache[bc] = _build_program(bc)
    return _prog_cache[bc]


def _q8(a):
    return np.clip(a, -240.0, 240.0).astype(NP_F8)


def _expand_rows(dense_block, sparse_block, ncols):
    """Place [13/26*100, ncols] blocks into the padded RTOT row layout."""
    out = np.zeros((RTOT, ncols), np.float32)
    out[1:1 + N_DENSE] = dense_block
    out[14:14 + N_DENSE] = dense_block / XLO_S
    out[SP0:NFEAT_END] = sparse_block
    return out


def _prep_shared(w_weight, w_bias, v):
    """vperm fp8 [P, NCHUNK, 2, CPAD] (same on every core).

    Row r = 256c + 128j + p holds feature row r of the padded table:
    r=0 bias, 1..13 v rows (paired with x_hi), 14..26 v/XLO_S (paired with
    the x residual rows), 32..2631 one-hot v rows.  Cols 0..623: field
    (col = k*39+f, scaled VSCALE); 624..639 V1hi = S1*sum_f v (s high);
    640..655 V1res (s residual, x S2M); 656/657 linear hi/res.
    """
    v = v.astype(np.float32)
    v2 = np.ascontiguousarray(v.transpose(0, 2, 1)).reshape(2613, CF)
    V1 = v.sum(axis=1)                  # [2613, 16]
    w = w_weight[0].astype(np.float32)

    field = _expand_rows(v2[:N_DENSE], v2[N_DENSE:], CF) * VSCALE
    v1f = _expand_rows(V1[:N_DENSE], V1[N_DENSE:], K_DIM) * S1
    wf = _expand_rows(w[:N_DENSE, None], w[N_DENSE:, None], 1) * S1
    wf[0, 0] = float(w_bias[0]) * S1

    vp8 = np.zeros((RTOT, CPAD), NP_F8)
    vp8[:, 0:CF] = _q8(field)
    hi = _q8(v1f)
    vp8[:, CF:CF + K_DIM] = hi
    vp8[:, CF + K_DIM:CF + 2 * K_DIM] = _q8(v1f - hi.astype(np.float32))
    whi = _q8(wf)
    vp8[:, NCOL - 2:NCOL - 1] = whi
    vp8[:, NCOL - 1:NCOL] = _q8(wf - whi.astype(np.float32))
    # [r, col] -> [p, c, j, col]
    return np.ascontiguousarray(
        vp8.reshape(NCHUNK, 2, P, CPAD).transpose(2, 0, 1, 3))


def _prep_core(x_core):
    """Per-core one-hot+dense fp8 [ngroups, P, NCHUNK, 2, gw] (as bytes)."""
    bc = x_core.shape[0]
    xd = x_core[:, :N_DENSE].astype(np.float32)
    x_hi8 = _q8(xd)
    x_lo8 = _q8((xd - x_hi8.astype(np.float32)) * XLO_S)
    one = np.float32(1.0).astype(NP_F8).view(np.uint8)

    oh = np.zeros((RTOT, bc), np.uint8)
    oh[0, :] = one
    oh[1:1 + N_DENSE] = x_hi8.view(np.uint8).T
    oh[14:14 + N_DENSE] = x_lo8.view(np.uint8).T
    idx = x_core[:, N_DENSE:].astype(np.int32)  # [bc, 26] in 0..99
    rows = SP0 + np.arange(N_SPARSE)[None, :] * SPARSE_DIM + idx  # [bc, 26]
    oh[rows, np.arange(bc)[:, None]] = one
    # [r, b] -> [g, p, c, j, gw]
    ngroups = bc // (GB * P)
    gw = GB * P
    oh = np.ascontiguousarray(
        oh.reshape(NCHUNK, 2, P, ngroups, gw).transpose(3, 2, 0, 1, 4))
    return oh.view(NP_F8)


def run(x, w_weight, w_bias, v, trace=False, trace_kwargs=None):
    x = np.asarray(x, np.float32)
    w_weight = np.asarray(w_weight, np.float32)
    w_bias = np.asarray(w_bias, np.float32)
    v = np.asarray(v, np.float32)
    assert x.shape == (B_FULL, 39), x.shape

    vp8 = _prep_shared(w_weight, w_bias, v)
    in_maps = []
    for i in range(N_CORES):
        xc = x[i * BC:(i + 1) * BC]
        in_maps.append({
            "oh": _prep_core(xc),
            "vperm": vp8,
        })

    nc = _get_program(BC)
    res = run_bass_kernel_spmd(
        nc, in_maps, list(range(N_CORES)),
        trace=trace, **(trace_kwargs or {}),
    )
    y = np.concatenate(
        [res.results[i]["y"].T.reshape(-1, 1) for i in range(N_CORES)], axis=0
    )
    return y.astype(np.float32), res


def kernel(x, w_weight, w_bias, v):
    y, _ = run(x, w_weight, w_bias, v)
    return y
